# revision 8
# baseline (speedup 1.0000x reference)
"""Trainium2 Bass kernel for a DP-GAT layer (dense masked attention), v9.

Computes, for x:[B,N,D], A_shape:[N,N] (0/1 adjacency), q,k,v:[D,D]:
    Q = x@q ; K = x@k
    S = Q @ K^T / sqrt(D)
    W = exp(8*tanh(S/8)) * A_shape
    out = (W / W.sum(-1, keepdims=True)) @ x @ v

Sharding: rows of N split across 8 NeuronCores (1024 rows each), SPMD,
no collectives. Host scatters inputs / gathers outputs.

Elementwise pipeline: exp(8*tanh(z)) = exp(16*sigmoid(2z) - 8), so one
Sigmoid table pass on ScalarE replaces tanh+exp, and the exp of the
non-negative sigmoid output is a Schraudolph bit-trick exp that the
row normalization makes exact up to a global power of two. The mask is
pre-scaled on the host to {0, 23637.1}, so ONE tensor_tensor multiply
fuses masking, the Schraudolph scale, and the fp->int16 convert:
    sig = Sigmoid(S^T * 1/(4*sqrt(D)))      ScalarE, PSUM->fp16
    w   = int16(sig * maskS)                VectorE tensor_tensor (2x mode)
    bitcast16(w) = 2^(23.083*sig - 15) = exp(16*sig) * 2^-15
2^-15 (and e^-8) cancels in W / W.sum(). Masked-out entries give
int16(0) = +0.0 exactly. The int16 tile feeds the PV matmuls directly
as an fp16 AP.

v15 on top of v13: batch 1's input loads move from one burst at unit-0
group 5 (inside the DMA-saturated jit mask-fill window) to one small
DMA per group across unit 0's back half; b2/b3's loads likewise
trickle one DMA per group through phase 1's last unit.

v17: the global PV lag is two groups deep (the sigmoid/mask chain gets
two groups of PE time before its PV fires, removing the ScalarE idle
at unit boundaries).

v16: the program's final group is emitted per-key-tile (sigmoid, mask
multiply and PV interleaved per j) so the end-of-kernel drain chain
pipelines instead of running three full-group stages back to back.

v13 on top of v12: ragged grouping reordered to [1]+[3]*21 so each
unit's first scores/sigmoid are one key-tile; startup loads the first
128 KT columns and the small first mask tile before everything else.

v12 on top of v11: mask-half refills are split across two units (11
tiles with the last consumer, 11 during the next phase's own first
batch, 11 groups ahead of their reader) to halve the refill bandwidth
spike; finer startup chunk interleave.

v11 on top of v10: the accumulator normalize is one strided
reciprocal + one broadcast tensor_tensor multiply (was 4 serial
tensor_scalar muls), shortening the acc-release chain that put a
~0.6-0.9us bubble at every (batch, i-chunk) boundary; deeper sig/w
pools absorb elementwise jitter.

v10 on top of v9: the PV lag crosses batch/phase boundaries (the PE
never drains between accumulators), and the startup interleaves the
first mask-cache tiles between batch 0's KT/XV chunks.

v9 structure: the adjacency mask is batch-invariant, so the loop runs
PAIR-MAJOR with a palindrome i-chunk order: (b0,b1)-ic0, (b0,b1)-ic1,
(b2,b3)-ic1, (b2,b3)-ic0. Each 512-query-row mask half is cached in
SBUF (22 group tiles, 64KB/partition) and reused by the two batches of
a phase, and the adjacent ic1 phases share one fill: total mask DMA
drops from 67MB (v2) / 42MB (v5-v8) to 25.2MB, removing the
DMA-bandwidth stalls that dominated the remaining gap (batch 0's
window needed 400GB/s in v8; phases need ~100GB/s). Only two batches'
KT/QT/XV are resident at a time (tile tags b%2 rotate the slots).

The D x D projections run on the HOST (0.006% of FLOPs): the device
receives KT [B,D,N] fp16, QT [B,D,RB] fp16 and XV=x@v+ones col
[B,128,NJT,130] fp8 ready-to-use.

Each group's PV matmuls are emitted one group late so the in-order PE
queue always holds the next group's score matmuls ahead of a PV that
still waits on its sigmoid/mask chain.

Device-side flow (per core), groups of 3 key-tiles (ragged 20x3+2x2):
    for half in (0, 1):           # 512 query rows each
      refill mask cache (interleaved with prior half's last batch)
      for b in 0..3:
        per group g:
          S^T  = KT_tile^T @ QT_chunk     -> PSUM [128, 3, 512] fp32
          sig  = Sigmoid(S^T/(4*sqrt(D))) -> SBUF fp16  (ScalarE)
          w    = int16(sig * maskS_g)     -> SBUF int16 (VectorE 2x)
          acc[i,0:129] += w16^T @ xv      -> PSUM (col 128 = rowsum via
                                             ones col; PV lagged 1 group)
        out = acc[:, :128] * (1/acc[:, 128])  -> fp16, one DMA

PSUM banks (8 x 2KB): score tiles 3 x 2 buffers + PV accumulator 2.
PE matmuls with start=True clear their entire output PSUM bank, so the
two acc slots sharing a bank are zeroed by the first PV matmul of each
bank (start=True) and all others accumulate.
"""

import math
import sys
from contextlib import ExitStack

import numpy as np

try:
    import concourse.bass as bass  # noqa: F401
except ImportError:  # pragma: no cover
    sys.path.insert(0, "/opt/trn_rl_repo")
    import concourse.bass as bass  # noqa: F401

import concourse.mybir as mybir
import concourse.tile as tile
from concourse import bacc
from concourse.bass_utils import run_bass_kernel_spmd

F32 = mybir.dt.float32
F16 = mybir.dt.float16
F8E4 = mybir.dt.float8e4
I16 = mybir.dt.int16

B, N, D = 4, 8192, 128
NCORES = 8
RB = N // NCORES  # query rows per core

IC = 512          # query-row chunk (free dim of score matmuls)
NIC = RB // IC    # i-chunks per core
NJT = N // 128    # key tiles total
# ragged key-tile grouping: 3-bank score tiles allow double buffering.
# The FIRST group is a single key-tile: at every (batch, i-chunk) boundary
# the next unit's first scores+sigmoid are tiny, so ScalarE restarts
# ~1us sooner (22 instructions either way -- no extra ACT overhead).
GS = [1] + [3] * 21              # group sizes, sum = NJT = 64
G0 = [sum(GS[:i]) for i in range(len(GS))]  # first key-tile of each group
NG = len(GS)
JGMAX = max(GS)

SIG_SCALE = 1.0 / (4.0 * math.sqrt(float(D)))
# Schraudolph: bitcast16(int16(sig*SCHRAUD)) = exp(16*sig) * 2^-15
SCHRAUD = 16.0 * 1024.0 / math.log(2.0)  # 23637.1


def build_program():
    nc = bacc.Bacc("TRN2", target_bir_lowering=False, debug=False)

    kt_d = nc.dram_tensor("ktr", [B, D, N], F16, kind="ExternalInput").ap()
    qt_d = nc.dram_tensor("qtr", [B, D, RB], F16, kind="ExternalInput").ap()
    xv_d = nc.dram_tensor("xvr", [B, 128, NJT, 130], F16, kind="ExternalInput").ap()
    maskR = nc.dram_tensor("maskR", [128, NIC, NJT, IC], F16, kind="ExternalInput").ap()
    out_d = nc.dram_tensor("out", [B, RB, D], F16, kind="ExternalOutput").ap()

    with tile.TileContext(nc) as tc, ExitStack() as ctx:
        mc_pool = ctx.enter_context(tc.tile_pool(name="mc", bufs=1))
        kt_pool = ctx.enter_context(tc.tile_pool(name="kt", bufs=1))
        qt_pool = ctx.enter_context(tc.tile_pool(name="qt", bufs=1))
        xv_pool = ctx.enter_context(tc.tile_pool(name="xv", bufs=1))
        sg_pool = ctx.enter_context(tc.tile_pool(name="sg", bufs=4))
        w_pool = ctx.enter_context(tc.tile_pool(name="w", bufs=5))
        ob_pool = ctx.enter_context(tc.tile_pool(name="ob", bufs=4))
        rs_pool = ctx.enter_context(tc.tile_pool(name="rs", bufs=4))
        st_ps = ctx.enter_context(tc.tile_pool(name="st_ps", bufs=2, space="PSUM"))
        acc_ps = ctx.enter_context(tc.tile_pool(name="acc_ps", bufs=1, space="PSUM"))

        tiles = {}   # b -> (kt, qt, xv)
        mcache = {}  # g -> current half's mask tile

        def load_batch(b):
            # chunked so round-robin spreads them across DMA queues and the
            # first groups unblock as soon as their columns land; tags b%2
            # rotate two resident slots (b2 reuses b0's, b3 reuses b1's)
            kt = kt_pool.tile([128, N], F16, tag=f"kt{b % 2}")
            qt = qt_pool.tile([128, RB], F16, tag=f"qt{b % 2}")
            xv = xv_pool.tile([128, NJT, 130], F16, tag=f"xv{b % 2}")
            tiles[b] = (kt, qt, xv)
            nc.sync.dma_start(qt[:], qt_d[b])
            for c in range(4):
                cw = N // 4
                nc.sync.dma_start(
                    kt[:, c * cw : (c + 1) * cw], kt_d[b][:, c * cw : (c + 1) * cw]
                )
            for c in range(2):
                cw = NJT // 2
                nc.sync.dma_start(
                    xv[:, c * cw : (c + 1) * cw], xv_d[b][:, c * cw : (c + 1) * cw]
                )

        def load_mcache_tile(ic, g):
            # one group tile of the current mask half; same tag across halves
            # so the pool rotation serializes the refill behind the previous
            # half's last reader
            t0, gs = G0[g], GS[g]
            mt = mc_pool.tile([128, gs, IC], F16, tag=f"mc{g}")
            nc.sync.dma_start(mt[:], maskR[:, ic, t0 : t0 + gs, :])
            mcache[g] = mt

        def group_scores(b, ic, g):
            kt, qt, xv = tiles[b]
            t0, gs = G0[g], GS[g]
            stp = st_ps.tile([128, JGMAX, IC], F32)
            for j in range(gs):
                nc.tensor.matmul(
                    stp[:, j],
                    kt[:, (t0 + j) * 128 : (t0 + j + 1) * 128],
                    qt[:, ic * IC : (ic + 1) * IC],
                    start=True, stop=True,
                )
            sig = sg_pool.tile([128, gs, IC], F16, tag="sig")
            nc.scalar.activation(
                sig[:], stp[:, 0:gs], mybir.ActivationFunctionType.Sigmoid,
                scale=SIG_SCALE,
            )
            w = w_pool.tile([128, gs, IC], I16, tag="w")
            nc.vector.tensor_mul(w[:], sig[:], mcache[g][:])
            return w

        def group_pv(b, g, w, acc):
            # emitted one group late so the PE always has the next group's
            # score matmuls queued ahead of a PV that waits on sigma/TT
            kt, qt, xv = tiles[b]
            t0, gs = G0[g], GS[g]
            wf = w[:].bitcast(F16)
            for j in range(gs):
                for s in range(IC // 128):
                    # start=True on the first matmul touching each acc bank
                    # clears the whole bank (two 256-col slots per bank).
                    nc.tensor.matmul(
                        acc[:, s * 256 : s * 256 + 129],
                        wf[:, j, s * 128 : (s + 1) * 128],
                        xv[:, t0 + j, 0:129],
                        start=(g == 0 and j == 0 and s % 2 == 0),
                        stop=(g == NG - 1 and j == gs - 1),
                        skip_group_check=True,
                    )

        def normalize(b, ic, acc):
            # one strided reciprocal + one broadcast multiply: shortens the
            # acc-release chain that gated the next unit's first PV matmul
            rs = rs_pool.tile([128, 4], F32)
            accv = acc[:].rearrange("p (s c) -> p s c", c=256)
            nc.vector.reciprocal(rs[:], accv[:, :, 128:129])
            ob = ob_pool.tile([128, 4, 128], F16)
            nc.vector.tensor_mul(
                ob[:], accv[:, :, 0:128], rs[:, :, None].broadcast_to([128, 4, 128])
            )
            nc.sync.dma_start(
                out_d[b, ic * IC : (ic + 1) * IC, :].rearrange(
                    "(s p) d -> p s d", p=128
                ),
                ob[:],
            )

        def batch_load_steps(b):
            # one small DMA per step so the loads trickle between groups
            kt = kt_pool.tile([128, N], F16, tag=f"kt{b % 2}")
            qt = qt_pool.tile([128, RB], F16, tag=f"qt{b % 2}")
            xv = xv_pool.tile([128, NJT, 130], F16, tag=f"xv{b % 2}")
            tiles[b] = (kt, qt, xv)
            steps = [lambda: nc.sync.dma_start(qt[:], qt_d[b])]
            for c in range(4):
                cw = N // 4
                steps.append(
                    lambda c=c, cw=cw: nc.sync.dma_start(
                        kt[:, c * cw : (c + 1) * cw],
                        kt_d[b][:, c * cw : (c + 1) * cw],
                    )
                )
            for c in range(4):
                cw = NJT // 4
                steps.append(
                    lambda c=c, cw=cw: nc.sync.dma_start(
                        xv[:, c * cw : (c + 1) * cw],
                        xv_d[b][:, c * cw : (c + 1) * cw],
                    )
                )
            return steps

        B1_STEPS = batch_load_steps(1)
        B23_STEPS = batch_load_steps(2) + batch_load_steps(3)

        # palindrome phase order: adjacent ic=1 phases share one mask fill
        phases = [((0, 1), 0), ((0, 1), 1), ((2, 3), 1), ((2, 3), 0)]
        # startup: interleave batch-0 chunks with the first mask tiles so
        # the first groups' masks don't queue behind all 4.3MB of batch 0
        kt0 = kt_pool.tile([128, N], F16, tag="kt0")
        qt0 = qt_pool.tile([128, RB], F16, tag="qt0")
        xv0 = xv_pool.tile([128, NJT, 130], F16, tag="xv0")
        tiles[0] = (kt0, qt0, xv0)
        nc.sync.dma_start(qt0[:], qt_d[0])
        nc.sync.dma_start(kt0[:, 0:128], kt_d[0][:, 0:128])
        load_mcache_tile(0, 0)
        nc.sync.dma_start(kt0[:, 128:1024], kt_d[0][:, 128:1024])
        load_mcache_tile(0, 1)
        nc.sync.dma_start(xv0[:, 0:8], xv_d[0][:, 0:8])
        load_mcache_tile(0, 2)
        nc.sync.dma_start(kt0[:, 1024:2048], kt_d[0][:, 1024:2048])
        load_mcache_tile(0, 3)
        load_mcache_tile(0, 4)
        nc.sync.dma_start(xv0[:, 8:24], xv_d[0][:, 8:24])
        nc.sync.dma_start(kt0[:, 2048:4096], kt_d[0][:, 2048:4096])
        for g in range(5, 10):
            load_mcache_tile(0, g)
        nc.sync.dma_start(xv0[:, 24:40], xv_d[0][:, 24:40])
        nc.sync.dma_start(kt0[:, 4096:6144], kt_d[0][:, 4096:6144])
        for g in range(10, 15):
            load_mcache_tile(0, g)
        nc.sync.dma_start(kt0[:, 6144:8192], kt_d[0][:, 6144:8192])
        nc.sync.dma_start(xv0[:, 40:64], xv_d[0][:, 40:64])
        for g in range(15, NG):
            load_mcache_tile(0, g)
        # the PV lag is global and TWO groups deep: the pending (w, acc)
        # entries cross batch and phase boundaries, and the sigmoid/mask
        # chain of group g has three groups of PE time before its PV fires
        pend = []  # [(b, ic, g, w, acc), ...] depth 2
        for pi, (pair, ic) in enumerate(phases):
            next_ic = phases[pi + 1][1] if pi + 1 < len(phases) else None
            for bi, b in enumerate(pair):
                acc = acc_ps.tile([128, 1024], F32)
                last_unit = pi == len(phases) - 1 and bi == 1
                # the program's very last group is emitted per-key-tile below
                for g in range(NG - 1 if last_unit else NG):
                    w = group_scores(b, ic, g)
                    if len(pend) == 3:
                        pb, pic, pg, pw, pacc = pend.pop(0)
                        group_pv(pb, pg, pw, pacc)
                        if pg == NG - 1:
                            normalize(pb, pic, pacc)
                    pend.append((b, ic, g, w, acc))
                    # b1's loads spread across unit 0's back half: the
                    # front half is already DMA-saturated by the jit mask
                    # fills (b1 isn't consumed until unit 1)
                    if pi == 0 and bi == 0 and g >= 12 and g < 12 + len(B1_STEPS):
                        B1_STEPS[g - 12]()
                    # b2/b3 loads spread once their b%2 slot frees mid-phase 1
                    if pi == 1 and bi == 1 and g >= 2 and g < 2 + len(B23_STEPS):
                        B23_STEPS[g - 2]()
                    # refill the cache with the next phase's half: first 11
                    # tiles during the last batch reading the current half,
                    # the rest during the next phase's first batch (11 groups
                    # ahead of their reader) -- halves the refill bandwidth
                    if next_ic is not None and next_ic != ic and bi == 1 and g < 11:
                        load_mcache_tile(next_ic, g)
                    if (
                        pi > 0
                        and phases[pi - 1][1] != ic
                        and bi == 0
                        and g < NG - 11
                    ):
                        load_mcache_tile(ic, g + 11)
        # flush the lagged PVs, then emit the final group per-key-tile
        # (sigmoid, mask, PV interleaved per j) so the end-of-kernel drain
        # chain pipelines instead of running three full-group stages
        # back to back
        while pend:
            pb, pic, pg, pw, pacc = pend.pop(0)
            group_pv(pb, pg, pw, pacc)
            if pg == NG - 1:
                normalize(pb, pic, pacc)
        g = NG - 1
        t0, gs = G0[g], GS[g]
        kt, qt, xv = tiles[pb]
        stp = st_ps.tile([128, JGMAX, IC], F32)
        for j in range(gs):
            nc.tensor.matmul(
                stp[:, j],
                kt[:, (t0 + j) * 128 : (t0 + j + 1) * 128],
                qt[:, pic * IC : (pic + 1) * IC],
                start=True, stop=True,
            )
        for j in range(gs):
            sigj = sg_pool.tile([128, 1, IC], F16, tag="sig")
            nc.scalar.activation(
                sigj[:], stp[:, j : j + 1],
                mybir.ActivationFunctionType.Sigmoid, scale=SIG_SCALE,
            )
            wj = w_pool.tile([128, 1, IC], I16, tag="w")
            nc.vector.tensor_mul(wj[:], sigj[:], mcache[g][:, j : j + 1])
            wjf = wj[:].bitcast(F16)
            for s in range(IC // 128):
                nc.tensor.matmul(
                    pacc[:, s * 256 : s * 256 + 129],
                    wjf[:, 0, s * 128 : (s + 1) * 128],
                    xv[:, t0 + j, 0:129],
                    start=False,
                    stop=(j == gs - 1),
                    skip_group_check=True,
                )
        normalize(pb, pic, pacc)

    nc.compile()
    return nc


_CACHED_NC = None


def _get_program():
    global _CACHED_NC
    if _CACHED_NC is None:
        _CACHED_NC = build_program()
    return _CACHED_NC


def make_in_maps(x, A_shape, q, k, v):
    x = np.ascontiguousarray(x, dtype=np.float32)
    q = np.ascontiguousarray(q, dtype=np.float32)
    k = np.ascontiguousarray(k, dtype=np.float32)
    v = np.ascontiguousarray(v, dtype=np.float32)

    # host projections, fp32 accumulate then low-precision cast
    K_all = np.einsum("bnd,de->bne", x, k)                  # [B, N, D]
    ktr = np.ascontiguousarray(K_all.transpose(0, 2, 1)).astype(np.float16)
    xv = np.einsum("bnd,de->bne", x, v)                     # [B, N, D]
    xvr_full = np.empty((B, N, 130), np.float32)
    xvr_full[:, :, :D] = xv
    xvr_full[:, :, D] = 1.0
    xvr_full[:, :, D + 1] = 0.0
    # [B, N, 130] -> [B, 128, NJT, 130]: row n = t*128 + p
    xvr = np.ascontiguousarray(
        xvr_full.reshape(B, NJT, 128, 130).transpose(0, 2, 1, 3)
).astype(np.float16)

    Q_all = np.einsum("bnd,de->bne", x, q)                  # [B, N, D]

    in_maps = []
    for c in range(NCORES):
        r0 = c * RB
        qtr = np.ascontiguousarray(
            Q_all[:, r0 : r0 + RB, :].transpose(0, 2, 1)
        ).astype(np.float16)
        # maskR[p, ic, t, ii] = SCHRAUD * A_shape[r0 + ic*IC + ii, t*128 + p]
        # (mask pre-scaled so one fp16 multiply performs mask + Schraudolph)
        maskR = np.ascontiguousarray(
            A_shape[r0 : r0 + RB, :]
            .T.reshape(NJT, 128, NIC, IC)
            .transpose(1, 2, 0, 3)
            * np.float32(SCHRAUD)
        ).astype(np.float16)
        in_maps.append(
            {"ktr": ktr, "qtr": qtr, "xvr": xvr, "maskR": maskR}
        )
    return in_maps


def kernel(x, A_shape, q, k, v):
    nc = _get_program()
    in_maps = make_in_maps(x, A_shape, q, k, v)
    res = run_bass_kernel_spmd(nc, in_maps, list(range(NCORES)))
    out = np.concatenate([res.results[c]["out"] for c in range(NCORES)], axis=1)
    return out.astype(np.float32)
